# revision 1
# baseline (speedup 1.0000x reference)
"""Conv2d-via-FFT reference implemented as a direct convolution on TRN2.

The reference pads to FFT size 61 >= 32+3-1, so its circular cross-correlation
equals the linear valid cross-correlation: out[n,f,i,j] =
sum_{c,p,q} x[n,c,i+p,j+q] * w[f,c,p,q] + bias[f].  That is an ordinary
stride-1 valid conv2d, which maps directly onto the PE array as 9 accumulated
matmuls (one per filter tap) with C=128 on the contraction partitions.

Sharding: data-parallel over N (64 samples -> 8 per core), filter replicated.
"""

import numpy as np

import concourse.bass as bass
import concourse.tile as tile
import concourse.bacc as bacc
import concourse.mybir as mybir
from concourse.bass_utils import run_bass_kernel_spmd

dt = mybir.dt
F32 = dt.float32
F32R = dt.float32r

N, C, H, W = 64, 128, 32, 32
F, KH, KW = 128, 3, 3
OH, OW = H - KH + 1, W - KW + 1          # 30, 30
NCORES = 8
NPC = N // NCORES                        # samples per core
ROWS_PER_CHUNK = 15                      # 2 chunks of 15 rows -> 450 px per matmul
NCHUNK = OH // ROWS_PER_CHUNK
CHUNK_PX = ROWS_PER_CHUNK * OW           # 450 <= 512 (one PSUM bank)


def _build():
    nc = bacc.Bacc("TRN2", target_bir_lowering=False, debug=False)

    x_d = nc.dram_tensor("x", [C, NPC, H, W], F32, kind="ExternalInput").ap()
    w_d = nc.dram_tensor("w", [C, KH * KW, F], F32, kind="ExternalInput").ap()
    b_d = nc.dram_tensor("bias", [F, 1], F32, kind="ExternalInput").ap()
    o_d = nc.dram_tensor("out", [NPC, F, OH * OW], F32, kind="ExternalOutput").ap()

    with tile.TileContext(nc) as tc:
        with tc.tile_pool(name="const", bufs=1) as const_pool, \
             tc.tile_pool(name="xp", bufs=1) as xp, \
             tc.tile_pool(name="ps", bufs=4, space="PSUM") as ps, \
             tc.tile_pool(name="ob", bufs=4) as ob:
            w_sb = const_pool.tile([C, KH * KW, F], F32R)
            nc.gpsimd.dma_start(w_sb[:], w_d[:])
            b_sb = const_pool.tile([F, 1], F32)
            nc.sync.dma_start(b_sb[:], b_d[:])

            x_sb = xp.tile([C, NPC, H, W], F32R)
            for n in range(NPC):
                nc.gpsimd.dma_start(x_sb[:, n], x_d[:, n])

            for n in range(NPC):
                for r in range(NCHUNK):
                    acc = ps.tile([F, CHUNK_PX], F32)
                    for k in range(KH * KW):
                        p, q = divmod(k, KW)
                        r0 = r * ROWS_PER_CHUNK + p
                        nc.tensor.matmul(
                            acc[:],
                            w_sb[:, k],
                            x_sb[:, n, r0:r0 + ROWS_PER_CHUNK, q:q + OW],
                            start=(k == 0),
                            stop=(k == KH * KW - 1),
                        )
                    o_sb = ob.tile([F, CHUNK_PX], F32)
                    nc.scalar.activation(
                        o_sb[:], acc[:],
                        mybir.ActivationFunctionType.Identity,
                        bias=b_sb[:],
                    )
                    nc.sync.dma_start(
                        o_d[n, :, r * CHUNK_PX:(r + 1) * CHUNK_PX], o_sb[:],
                    )

    nc.compile()
    return nc


_NC = None


def _get_nc():
    global _NC
    if _NC is None:
        _NC = _build()
    return _NC


def _in_maps(x, w, bias):
    w_prep = np.ascontiguousarray(
        w.transpose(1, 2, 3, 0).reshape(C, KH * KW, F).astype(np.float32))
    b_prep = np.ascontiguousarray(bias.astype(np.float32).reshape(F, 1))
    maps = []
    for c in range(NCORES):
        xc = np.ascontiguousarray(
            x[c * NPC:(c + 1) * NPC].transpose(1, 0, 2, 3).astype(np.float32))
        maps.append({"x": xc, "w": w_prep, "bias": b_prep})
    return maps


def run(x, w, bias, trace=False, **spmd_kwargs):
    """Run the SPMD kernel; returns (out [N,F,OH,OW], BassKernelResults)."""
    nc = _get_nc()
    res = run_bass_kernel_spmd(nc, _in_maps(x, w, bias), list(range(NCORES)),
                               trace=trace, **spmd_kwargs)
    parts = [res.results[c]["out"].reshape(NPC, F, OH, OW) for c in range(NCORES)]
    return np.concatenate(parts, axis=0), res


def kernel(x, w, bias):
    out, _ = run(np.asarray(x), np.asarray(w), np.asarray(bias))
    return out


# revision 2
# speedup vs baseline: 1.0023x; 1.0023x over previous
"""Conv2d-via-FFT reference implemented as a direct convolution on TRN2.

The reference pads to FFT size 61 >= 32+3-1, so its circular cross-correlation
equals the linear valid cross-correlation: out[n,f,i,j] =
sum_{c,p,q} x[n,c,i+p,j+q] * w[f,c,p,q] + bias[f].  That is an ordinary
stride-1 valid conv2d, which maps directly onto the PE array as 9 accumulated
matmuls (one per filter tap) with C=128 on the contraction partitions.

Sharding: data-parallel over N (64 samples -> 8 per core), filter replicated.
"""

import numpy as np

import concourse.bass as bass
import concourse.tile as tile
import concourse.bacc as bacc
import concourse.mybir as mybir
from concourse.bass_utils import run_bass_kernel_spmd

dt = mybir.dt
F32 = dt.float32
F32R = dt.float32r

N, C, H, W = 64, 128, 32, 32
F, KH, KW = 128, 3, 3
OH, OW = H - KH + 1, W - KW + 1          # 30, 30
NCORES = 8
NPC = N // NCORES                        # samples per core
ROWS_PER_CHUNK = 15                      # 2 chunks of 15 rows -> 450 px per matmul
NCHUNK = OH // ROWS_PER_CHUNK
CHUNK_PX = ROWS_PER_CHUNK * OW           # 450 <= 512 (one PSUM bank)


def _build():
    nc = bacc.Bacc("TRN2", target_bir_lowering=False, debug=False)

    # x and w are declared float32r: raw fp32 bits are fed straight to the
    # PE's reduced-precision fp32 path via fast HWDGE DMAs (no cast DMA).
    x_d = nc.dram_tensor("x", [C, NPC, H, W], F32R, kind="ExternalInput").ap()
    w_d = nc.dram_tensor("w", [C, KH * KW, F], F32R, kind="ExternalInput").ap()
    b_d = nc.dram_tensor("bias", [F, 1], F32, kind="ExternalInput").ap()
    o_d = nc.dram_tensor("out", [NPC, F, OH * OW], F32, kind="ExternalOutput").ap()

    with tile.TileContext(nc) as tc:
        with tc.tile_pool(name="const", bufs=1) as const_pool, \
             tc.tile_pool(name="xp", bufs=3) as xp, \
             tc.tile_pool(name="ps", bufs=4, space="PSUM") as ps, \
             tc.tile_pool(name="ob", bufs=4) as ob:
            w_sb = const_pool.tile([C, KH * KW, F], F32R)
            nc.sync.dma_start(w_sb[:], w_d[:])
            b_sb = const_pool.tile([F, 1], F32)
            nc.sync.dma_start(b_sb[:], b_d[:])

            for n in range(NPC):
                x_sb = xp.tile([C, H, W], F32R, tag="x")
                nc.sync.dma_start(x_sb[:], x_d[:, n])
                for r in range(NCHUNK):
                    acc = ps.tile([F, CHUNK_PX], F32)
                    for k in range(KH * KW):
                        p, q = divmod(k, KW)
                        r0 = r * ROWS_PER_CHUNK + p
                        nc.tensor.matmul(
                            acc[:],
                            w_sb[:, k],
                            x_sb[:, r0:r0 + ROWS_PER_CHUNK, q:q + OW],
                            start=(k == 0),
                            stop=(k == KH * KW - 1),
                        )
                    o_sb = ob.tile([F, CHUNK_PX], F32)
                    nc.scalar.activation(
                        o_sb[:], acc[:],
                        mybir.ActivationFunctionType.Identity,
                        bias=b_sb[:],
                    )
                    nc.sync.dma_start(
                        o_d[n, :, r * CHUNK_PX:(r + 1) * CHUNK_PX], o_sb[:],
                    )

    nc.compile()
    return nc


_NC = None


def _get_nc():
    global _NC
    if _NC is None:
        _NC = _build()
    return _NC


def _in_maps(x, w, bias):
    w_prep = np.ascontiguousarray(
        w.transpose(1, 2, 3, 0).reshape(C, KH * KW, F).astype(np.float32))
    b_prep = np.ascontiguousarray(bias.astype(np.float32).reshape(F, 1))
    maps = []
    for c in range(NCORES):
        xc = np.ascontiguousarray(
            x[c * NPC:(c + 1) * NPC].transpose(1, 0, 2, 3).astype(np.float32))
        maps.append({"x": xc, "w": w_prep, "bias": b_prep})
    return maps


def run(x, w, bias, trace=False, **spmd_kwargs):
    """Run the SPMD kernel; returns (out [N,F,OH,OW], BassKernelResults)."""
    nc = _get_nc()
    res = run_bass_kernel_spmd(nc, _in_maps(x, w, bias), list(range(NCORES)),
                               trace=trace, **spmd_kwargs)
    parts = [res.results[c]["out"].reshape(NPC, F, OH, OW) for c in range(NCORES)]
    return np.concatenate(parts, axis=0), res


def kernel(x, w, bias):
    out, _ = run(np.asarray(x), np.asarray(w), np.asarray(bias))
    return out


# revision 4
# speedup vs baseline: 1.0204x; 1.0181x over previous
"""Conv2d-via-FFT reference implemented as a direct convolution on TRN2.

The reference pads to FFT size 61 >= 32+3-1, so its circular cross-correlation
equals the linear valid cross-correlation: out[n,f,i,j] =
sum_{c,p,q} x[n,c,i+p,j+q] * w[f,c,p,q] + bias[f].  That is an ordinary
stride-1 valid conv2d, which maps directly onto the PE array as 9 accumulated
matmuls (one per filter tap) with C=128 on the contraction partitions.

Sharding: data-parallel over N (64 samples -> 8 per core), filter replicated.
"""

import numpy as np

import concourse.bass as bass
import concourse.tile as tile
import concourse.bacc as bacc
import concourse.mybir as mybir
from concourse.bass_utils import run_bass_kernel_spmd

dt = mybir.dt
F32 = dt.float32
F32R = dt.float32r

N, C, H, W = 64, 128, 32, 32
F, KH, KW = 128, 3, 3
OH, OW = H - KH + 1, W - KW + 1          # 30, 30
NCORES = 8
NPC = N // NCORES                        # samples per core
ROWS_PER_CHUNK = 15                      # 2 chunks of 15 rows -> 450 px per matmul
NCHUNK = OH // ROWS_PER_CHUNK
CHUNK_PX = ROWS_PER_CHUNK * OW           # 450 <= 512 (one PSUM bank)


def _build():
    nc = bacc.Bacc("TRN2", target_bir_lowering=False, debug=False)

    # x and w are declared float32r: raw fp32 bits are fed straight to the
    # PE's reduced-precision fp32 path via fast HWDGE DMAs (no cast DMA).
    x_d = nc.dram_tensor("x", [C, NPC, H, W], F32R, kind="ExternalInput").ap()
    w_d = nc.dram_tensor("w", [C, KH * KW, F], F32R, kind="ExternalInput").ap()
    b_d = nc.dram_tensor("bias", [F, 1], F32, kind="ExternalInput").ap()
    o_d = nc.dram_tensor("out", [NPC, F, OH * OW], F32, kind="ExternalOutput").ap()

    with tile.TileContext(nc) as tc:
        with tc.tile_pool(name="const", bufs=1) as const_pool, \
             tc.tile_pool(name="xp", bufs=3) as xp, \
             tc.tile_pool(name="ps", bufs=4, space="PSUM") as ps, \
             tc.tile_pool(name="ob", bufs=4) as ob:
            w_sb = const_pool.tile([C, KH * KW, F], F32R)
            nc.sync.dma_start(w_sb[:], w_d[:])
            b_sb = const_pool.tile([F, 1], F32)
            nc.scalar.dma_start(b_sb[:], b_d[:])

            # Split each sample's load into row halves so chunk-0 matmuls can
            # start as soon as rows 0..16 land; triggers alternate between the
            # Sync and Vector queues to avoid serializing on one engine.
            x_tiles = []
            for n in range(NPC):
                x_sb = xp.tile([C, H, W], F32R, tag="x", name=f"x_sb{n}")
                eng = nc.sync if n % 2 == 0 else nc.scalar
                eng.dma_start(x_sb[:, 0:17], x_d[:, n, 0:17])
                eng.dma_start(x_sb[:, 17:32], x_d[:, n, 17:32])
                x_tiles.append(x_sb)

            for n in range(NPC):
                x_sb = x_tiles[n]
                for r in range(NCHUNK):
                    acc = ps.tile([F, CHUNK_PX], F32)
                    for k in range(KH * KW):
                        p, q = divmod(k, KW)
                        r0 = r * ROWS_PER_CHUNK + p
                        nc.tensor.matmul(
                            acc[:],
                            w_sb[:, k],
                            x_sb[:, r0:r0 + ROWS_PER_CHUNK, q:q + OW],
                            start=(k == 0),
                            stop=(k == KH * KW - 1),
                        )
                    o_sb = ob.tile([F, CHUNK_PX], F32)
                    nc.scalar.activation(
                        o_sb[:], acc[:],
                        mybir.ActivationFunctionType.Identity,
                        bias=b_sb[:],
                    )
                    nc.sync.dma_start(
                        o_d[n, :, r * CHUNK_PX:(r + 1) * CHUNK_PX], o_sb[:],
                    )

    nc.compile()
    return nc


_NC = None


def _get_nc():
    global _NC
    if _NC is None:
        _NC = _build()
    return _NC


def _in_maps(x, w, bias):
    w_prep = np.ascontiguousarray(
        w.transpose(1, 2, 3, 0).reshape(C, KH * KW, F).astype(np.float32))
    b_prep = np.ascontiguousarray(bias.astype(np.float32).reshape(F, 1))
    maps = []
    for c in range(NCORES):
        xc = np.ascontiguousarray(
            x[c * NPC:(c + 1) * NPC].transpose(1, 0, 2, 3).astype(np.float32))
        maps.append({"x": xc, "w": w_prep, "bias": b_prep})
    return maps


def run(x, w, bias, trace=False, **spmd_kwargs):
    """Run the SPMD kernel; returns (out [N,F,OH,OW], BassKernelResults)."""
    nc = _get_nc()
    res = run_bass_kernel_spmd(nc, _in_maps(x, w, bias), list(range(NCORES)),
                               trace=trace, **spmd_kwargs)
    parts = [res.results[c]["out"].reshape(NPC, F, OH, OW) for c in range(NCORES)]
    return np.concatenate(parts, axis=0), res


def kernel(x, w, bias):
    out, _ = run(np.asarray(x), np.asarray(w), np.asarray(bias))
    return out
